# revision 1
# baseline (speedup 1.0000x reference)
"""Trainium2 Bass kernel for the LIIF non-parametric per-pixel mini-MLP.

Reference computation (per branch, per pixel p = (b,h,w)):
    channels c of feat reshape to W[head, o, i] with c = head*64 + o*8 + i
    t[T, i] = t_coord[T]  (broadcast over i)
    h = einsum('OI,TI->TO', W0, t);  then for k in 1..3: h = W_k @ relu(h)
    out[T] = h[T, 0]

Key algebraic identity used here: since t enters rank-1 in T and
relu(s*t) = relu(s)*relu(t) + relu(-s)*relu(-t) (disjoint support in t),
every intermediate stays in span{u, v} with u = relu(t), v = relu(-t):
    s0[i]  = sum_j W0[i, j]
    a1 = relu(s0),            b1 = relu(-s0)
    a2 = relu(W1 @ a1),       b2 = relu(W1 @ b1)
    a3 = relu(W2 @ a2),       b3 = relu(W2 @ b2)
    alpha = W3[0, :] . a3,    beta = W3[0, :] . b3
    out[T] = alpha * u[T] + beta * v[T]
Only channels 0:200 of the 256 are ever needed (row 0 of W3).

On-chip mapping (per unit = 512 pixels x both branches; [partition, free]):
    F012 [128, 1536]: partitions 0:64 = x_real channels, 64:128 = x_imag
                      free: 3 groups of 512 px for channel-groups c0:64,
                      c64:128, c128:192 (channel g*64+p at free group g)
    X1 = CM1^T @ F0      (PE)   s0 replicated to all (o,i) slots, both branches
    P1a = max(X1,0)*F1   (DVE scalar_tensor_tensor, fused relu+mult)
    P1b = min(X1,0)*F1   (DVE)  equals -relu(-s0)*W1; sign fixed by CM1n
    X2a = CM1^T @ P1a,  X2b = CM1n^T @ P1b   (PE)
    P2a = max(X2a,0)*F2, P2b = max(X2b,0)*F2 (DVE)
    X3[32,512] = C3a^T @ P2a + C3b^T @ P2b   (PE, accumulated)
                 rows: [a3_re, b3_re, a3_im, b3_im] pre-relu
    P3 = max(X3,0)*F34   (DVE)  F34 = W3row0 repeated [re,re,im,im]
    OUT[128,512] = G2^T @ P3    (PE)  partitions = (branch, T), rank-2 expansion
    copy PSUM->SBUF (ACT), DMA out.

Sharding: 8 cores, core k -> batch b = k//2, h-half = k%2 (64 h-rows each).
"""

import os
import numpy as np

import concourse.bass as bass
import concourse.bacc as bacc
import concourse.tile as tile
from concourse import mybir
from concourse import bass_utils

F32 = mybir.dt.float32

NUM_CORES = 8
C_USED = 200          # channels actually needed
H_SH = 64             # h rows per core
W_ = 128
T_ = 64
N_UNITS = 16          # units per core; each unit covers 4 h rows = 512 px
PX = 512              # pixels per unit

# Matmul input dtype: float32r runs the PE at 1 cycle/column instead of 4.
# Verified on hardware to be bit-identical to float32 for these matmuls
# (see MM_DTYPE sweep in development); can be flipped back via env var.
USE_F32R = os.environ.get("KERNEL_MM_F32", "0") != "1"


def _build_const_mats(t_coord: np.ndarray):
    """Host-side constant matrices (tiny, derived from fixed structure + t_coord)."""
    # M1[k = 8i+j, m = 8o+i] = 1 : rep-reduce within one branch block
    m1 = np.zeros((64, 64), np.float32)
    for o in range(8):
        for i in range(8):
            for j in range(8):
                m1[8 * i + j, 8 * o + i] = 1.0
    cm1 = np.zeros((128, 128), np.float32)
    cm1[0:64, 0:64] = m1
    cm1[64:128, 64:128] = m1
    cm1n = -cm1

    # C3a/C3b [128, 32]: reduce products to X3 rows [a_re, b_re, a_im, b_im]
    c3a = np.zeros((128, 32), np.float32)
    c3b = np.zeros((128, 32), np.float32)
    for i in range(8):
        for j in range(8):
            c3a[8 * i + j, i] = 1.0            # a3_re from P2a re-half
            c3a[64 + 8 * i + j, 16 + i] = 1.0  # a3_im from P2a im-half
            c3b[8 * i + j, 8 + i] = 1.0        # b3_re from P2b re-half
            c3b[64 + 8 * i + j, 24 + i] = 1.0  # b3_im from P2b im-half

    # G2 [32, 128]: rank-2 expansion. row 8*(2*br + s) + i, col 64*br + T
    t = t_coord.astype(np.float32)
    u = np.maximum(t, 0.0)
    v = np.maximum(-t, 0.0)
    g2 = np.zeros((32, 128), np.float32)
    for br in range(2):
        for i in range(8):
            g2[8 * (2 * br + 0) + i, 64 * br:64 * (br + 1)] = u
            g2[8 * (2 * br + 1) + i, 64 * br:64 * (br + 1)] = v
    return cm1, cm1n, c3a, c3b, g2


def _build_program():
    # Matmul-side dtype. float32r is fp32 with an 11-bit mantissa (low 12 bits
    # dropped by the PE), running the array at 1 cycle/column instead of 4.
    # The walrus verifier requires every fp32r-matmul input's producer to
    # declare fp32r output, so the DRAM tensors, F tiles, and product tiles
    # are all declared fp32r; the DVE reads the F tiles via an f32 bitcast
    # (any fp32r pattern is a valid fp32).
    MMDT = mybir.dt.float32r if USE_F32R else F32

    nc = bacc.Bacc("TRN2", target_bir_lowering=False, debug=False,
                   enable_asserts=False)
    # Inputs are pre-arranged host-side so every tile load is ONE <=3-dim DMA
    # (fewer DMA semaphores per consuming matmul; the self-loading matmul has
    # a tight HW sync-wait budget).
    # xp[p, g, h, w] = x[br, g*64+c, h, w] with p = 64*br + c  (channels 0:192)
    # xt[q, h, w]    = x[br, 192+c, h, w] with q = 16*br + 8*dup + c
    xp_d = nc.dram_tensor("xp", [128, 3, H_SH, W_], MMDT, kind="ExternalInput").ap()
    xt_d = nc.dram_tensor("xt", [32, H_SH, W_], MMDT, kind="ExternalInput").ap()
    cmats_d = nc.dram_tensor("cmats", [128, 448], MMDT, kind="ExternalInput").ap()
    out_d = nc.dram_tensor("out", [2, T_, H_SH, W_], F32, kind="ExternalOutput").ap()

    MAX_ = mybir.AluOpType.max
    MIN_ = mybir.AluOpType.min
    MULT = mybir.AluOpType.mult

    def mm(out, lhsT, rhs, **kw):
        nc.tensor.matmul(out, lhsT, rhs, **kw)

    def as_f32(ap):
        return ap.bitcast(F32) if USE_F32R else ap

    with tile.TileContext(nc) as tc:
        with (
            tc.tile_pool(name="consts", bufs=1) as consts,
            tc.tile_pool(name="fpool", bufs=3) as fpool,
            tc.tile_pool(name="ppool", bufs=2) as ppool,
            tc.tile_pool(name="opool", bufs=3) as opool,
            tc.tile_pool(name="psum", bufs=1, space="PSUM") as psum,
        ):
            CT = consts.tile([128, 448], MMDT)
            nc.sync.dma_start(out=CT, in_=cmats_d)
            CM1 = CT[:, 0:128]
            CM1N = CT[:, 128:256]
            C3A = CT[:, 256:288]
            C3B = CT[:, 288:320]
            G2 = CT[0:32, 320:448]

            o_tiles = []
            for uidx in range(N_UNITS):
                hl = 4 * uidx
                # ---- loads ----
                F012 = fpool.tile([128, 3, PX], MMDT, tag="F012")
                nc.sync.dma_start(out=F012, in_=xp_d[:, :, hl:hl + 4, :])
                F34 = fpool.tile([32, PX], MMDT, tag="F34")
                nc.sync.dma_start(out=F34, in_=xt_d[:, hl:hl + 4, :])

                # ---- layer 0: s0 replicated ----
                X1 = psum.tile([128, PX], F32, tag="X1", bufs=2)
                mm(X1, CM1, F012[:, 0, :])

                # ---- layer 1 products (fused relu via max/min with 0) ----
                P1a = ppool.tile([128, PX], MMDT, tag="P1a")
                nc.vector.scalar_tensor_tensor(
                    out=P1a, in0=X1, scalar=0.0, in1=as_f32(F012[:, 1, :]),
                    op0=MAX_, op1=MULT)
                P1b = ppool.tile([128, PX], MMDT, tag="P1b")
                nc.vector.scalar_tensor_tensor(
                    out=P1b, in0=X1, scalar=0.0, in1=as_f32(F012[:, 1, :]),
                    op0=MIN_, op1=MULT)

                X2a = psum.tile([128, PX], F32, tag="X2a")
                mm(X2a, CM1, P1a)
                X2b = psum.tile([128, PX], F32, tag="X2b")
                mm(X2b, CM1N, P1b)

                # ---- layer 2 products ----
                P2a = ppool.tile([128, PX], MMDT, tag="P2a")
                nc.vector.scalar_tensor_tensor(
                    out=P2a, in0=X2a, scalar=0.0, in1=as_f32(F012[:, 2, :]),
                    op0=MAX_, op1=MULT)
                P2b = ppool.tile([128, PX], MMDT, tag="P2b")
                nc.vector.scalar_tensor_tensor(
                    out=P2b, in0=X2b, scalar=0.0, in1=as_f32(F012[:, 2, :]),
                    op0=MAX_, op1=MULT)

                # ---- layer 3 reduce into [a3_re, b3_re, a3_im, b3_im] ----
                X3 = psum.tile([32, PX], F32, tag="X3")
                mm(X3, C3A, P2a, start=True, stop=False)
                mm(X3, C3B, P2b, start=False, stop=True)

                P3 = ppool.tile([32, PX], MMDT, tag="P3")
                nc.vector.scalar_tensor_tensor(
                    out=P3, in0=X3, scalar=0.0, in1=as_f32(F34), op0=MAX_, op1=MULT)

                # ---- rank-2 expansion over (branch, T) ----
                XO = psum.tile([128, PX], F32, tag="XO", bufs=2)
                mm(XO, G2, P3)

                O = opool.tile([128, PX], F32, tag="O")
                nc.scalar.copy(O, XO)
                o_tiles.append(O)
                nc.scalar.dma_start(out=out_d[:, :, hl:hl + 4, :], in_=O)
    nc.compile()
    return nc


_PROGRAM_CACHE = {}


def _get_program():
    key = ("f32r" if USE_F32R else "f32",)
    if key not in _PROGRAM_CACHE:
        _PROGRAM_CACHE[key] = _build_program()
    return _PROGRAM_CACHE[key]


def _make_in_maps(x_real, x_imag, t_coord):
    cm1, cm1n, c3a, c3b, g2 = _build_const_mats(np.asarray(t_coord))
    cmats = np.zeros((128, 448), np.float32)
    cmats[:, 0:128] = cm1
    cmats[:, 128:256] = cm1n
    cmats[:, 256:288] = c3a
    cmats[:, 288:320] = c3b
    cmats[0:32, 320:448] = g2
    x_real = np.asarray(x_real)
    x_imag = np.asarray(x_imag)
    in_maps = []
    for core in range(NUM_CORES):
        b = core // 2
        h0 = H_SH * (core % 2)
        xs = np.stack([
            x_real[b, 0:192, h0:h0 + H_SH, :],
            x_imag[b, 0:192, h0:h0 + H_SH, :],
        ])  # [2, 192, H, W]
        # xp[(br, c), g, h, w] = xs[br, g*64+c, h, w]
        xp = np.ascontiguousarray(
            xs.reshape(2, 3, 64, H_SH, W_).transpose(0, 2, 1, 3, 4)
            .reshape(128, 3, H_SH, W_))
        x3r = x_real[b, 192:200, h0:h0 + H_SH, :]
        x3i = x_imag[b, 192:200, h0:h0 + H_SH, :]
        xt = np.ascontiguousarray(
            np.stack([x3r, x3r, x3i, x3i]).reshape(32, H_SH, W_))
        in_maps.append({"xp": xp, "xt": xt, "cmats": cmats})
    return in_maps


def _assemble(results):
    out = np.empty((2, 4, T_, 128, W_), np.float32)
    for core in range(NUM_CORES):
        b = core // 2
        h0 = H_SH * (core % 2)
        out[:, b, :, h0:h0 + H_SH, :] = results[core]["out"]
    return out


def kernel_with_info(x_real, x_imag, t_coord, trace=False):
    nc = _get_program()
    in_maps = _make_in_maps(x_real, x_imag, t_coord)
    res = bass_utils.run_bass_kernel_spmd(
        nc, in_maps, core_ids=list(range(NUM_CORES)), trace=trace)
    return _assemble(res.results), res


def kernel(x_real, x_imag, t_coord):
    out, _ = kernel_with_info(x_real, x_imag, t_coord)
    return out



# revision 4
# speedup vs baseline: 1.3386x; 1.3386x over previous
"""Trainium2 Bass kernel for the LIIF non-parametric per-pixel mini-MLP.

Reference computation (per branch, per pixel p = (b,h,w)):
    channels c of feat reshape to W[head, o, i] with c = head*64 + o*8 + i
    t[T, i] = t_coord[T]  (broadcast over i)
    h = einsum('OI,TI->TO', W0, t);  then for k in 1..3: h = W_k @ relu(h)
    out[T] = h[T, 0]

Algebraic identity: t enters rank-1 in T and relu(s*t) = relu(s)*relu(t) +
relu(-s)*relu(-t) (disjoint support), so every intermediate stays in
span{u, v} with u = relu(t), v = relu(-t):
    s0[i]  = sum_j W0[i, j]
    a1 = relu(s0),            b1 = relu(-s0)
    a2 = relu(W1 @ a1),       b2 = relu(W1 @ b1)
    a3 = relu(W2 @ a2),       b3 = relu(W2 @ b2)
    alpha = W3[0, :] . a3,    beta = W3[0, :] . b3
    out[T] = alpha * u[T] + beta * v[T]
Only channels 0:200 of 256 are needed (row 0 of W3).

On-chip mapping (per unit = 512 pixels x both branches; [partition, free]):
    F0/F1/F2 [128, 512] views of a big SBUF tensor (chunked bulk DMA):
        partitions 0:64 = x_real channel slots, 64:128 = x_imag;
        channel g*64 + p%64 at group g.
    X1 = CM1^T @ F0      (PE)   s0 replicated to all (o,i) slots, both branches
    P1a = max(X1,0)*F1   (DVE)  fused relu+mult
    P1b = min(X1,0)*F1   (DVE)  stored = -true_P1b
    X2a = CM1^T @ P1a, X2b = CM1^T @ P1b   (PE; X2b stored = -true)
    P2a = max(X2a,0)*F2, P2b = min(X2b,0)*F2  (DVE; P2b stored = -true)
    X3[32,512] = C3A^T @ P2a + C3BN^T @ P2b   (PE accum; C3BN = -C3B fixes sign)
    P3 = max(X3,0)*F34   (DVE)  F34 = W3row0 repeated [re,re,im,im]
    XO[128,512] = G2^T @ P3    (PE)  partitions = (branch, T), rank-2 expansion
    O = copy(XO) -> bf16 SBUF (ACT), chunked DMA out.

All matmul operands are bf16 (PE runs 1 cycle/column vs 4 for fp32) and all
HBM traffic is bf16 (host converts); PSUM stays fp32. Every stationary except
CM1N is eliminated so the per-unit weight sequence is CM1,CM1,CM1,C3A,C3BN,G2.

The PE instruction stream is software-pipelined 3 rounds deep
(X1(r+2), X2ab(r+1), X3ab(r), XO(r-1)) so no matmul ever waits on a DVE
result produced in the same round.

Sharding: 8 cores, core k -> batch b = k//2, h-half = k%2 (64 h-rows each).
"""

import numpy as np
import ml_dtypes

import concourse.bass as bass
import concourse.bacc as bacc
import concourse.tile as tile
from concourse import mybir
from concourse import bass_utils

F32 = mybir.dt.float32
BF16 = mybir.dt.bfloat16
NPBF16 = np.dtype(ml_dtypes.bfloat16)

NUM_CORES = 8
H_SH = 64             # h rows per core
W_ = 128
T_ = 64
N_UNITS = 16          # units per core; each unit covers 4 h rows = 512 px
PX = 512              # pixels per unit
CHUNKS = [2, 2, 4, 4, 4]          # units per input-DMA chunk (ramp-friendly)
CHUNK_START = [0, 2, 4, 8, 12]
OUT_CHUNK = 4                     # units per output DMA


def _build_const_mats(t_coord: np.ndarray):
    """Host-side constant matrices (tiny, derived from fixed structure + t_coord)."""
    # M1[k = 8i+j, m = 8o+i] = 1 : rep-reduce within one branch block
    m1 = np.zeros((64, 64), np.float32)
    for o in range(8):
        for i in range(8):
            for j in range(8):
                m1[8 * i + j, 8 * o + i] = 1.0
    cm1 = np.zeros((128, 128), np.float32)
    cm1[0:64, 0:64] = m1
    cm1[64:128, 64:128] = m1

    # C3A/C3BN [128, 32]: reduce products to X3 rows [a_re, b_re, a_im, b_im].
    # C3BN carries a -1 so the sign-inverted b-stream (stored = -true) lands
    # with the correct sign in the accumulated X3.
    c3a = np.zeros((128, 32), np.float32)
    c3bn = np.zeros((128, 32), np.float32)
    for i in range(8):
        for j in range(8):
            c3a[8 * i + j, i] = 1.0              # a3_re from P2a re-half
            c3a[64 + 8 * i + j, 16 + i] = 1.0    # a3_im from P2a im-half
            c3bn[8 * i + j, 8 + i] = -1.0        # b3_re from P2b re-half
            c3bn[64 + 8 * i + j, 24 + i] = -1.0  # b3_im from P2b im-half

    # G2 [32, 128]: rank-2 expansion. row 8*(2*br + s) + i, col 64*br + T
    t = t_coord.astype(np.float32)
    u = np.maximum(t, 0.0)
    v = np.maximum(-t, 0.0)
    g2 = np.zeros((32, 128), np.float32)
    for br in range(2):
        for i in range(8):
            g2[8 * (2 * br + 0) + i, 64 * br:64 * (br + 1)] = u
            g2[8 * (2 * br + 1) + i, 64 * br:64 * (br + 1)] = v
    return cm1, c3a, c3bn, g2


def _build_program():
    nc = bacc.Bacc("TRN2", target_bir_lowering=False, debug=False,
                   enable_asserts=False)
    # xp[p, u, g, px] = x[br, g*64 + c, 4u + px//128, px%128], p = 64*br + c
    xp_d = nc.dram_tensor("xp", [128, N_UNITS, 3, PX], BF16, kind="ExternalInput").ap()
    # xt[q, u, px]: q = 16*br_pair + 8*dup + c for channels 192:200, [re,re,im,im]
    xt_d = nc.dram_tensor("xt", [32, N_UNITS, PX], BF16, kind="ExternalInput").ap()
    cmats_d = nc.dram_tensor("cmats", [128, 320], BF16, kind="ExternalInput").ap()
    out_d = nc.dram_tensor("out", [128, N_UNITS, PX], BF16, kind="ExternalOutput").ap()

    MAX_ = mybir.AluOpType.max
    MIN_ = mybir.AluOpType.min
    MULT = mybir.AluOpType.mult

    mm = nc.tensor.matmul
    stt = nc.vector.scalar_tensor_tensor

    with tile.TileContext(nc) as tc:
        with (
            tc.tile_pool(name="consts", bufs=1) as consts,
            tc.tile_pool(name="xpool", bufs=1) as xpool,
            tc.tile_pool(name="ppool", bufs=2) as ppool,
            tc.tile_pool(name="opool", bufs=2) as opool,
            tc.tile_pool(name="psum", bufs=1, space="PSUM") as psum,
        ):
            CT = consts.tile([128, 320], BF16)
            nc.sync.dma_start(out=CT, in_=cmats_d)
            CM1 = CT[:, 0:128]
            C3A = CT[:, 128:160]
            C3BN = CT[:, 160:192]
            G2 = CT[0:32, 192:320]

            xp_tiles = []
            for c, (n, s) in enumerate(zip(CHUNKS, CHUNK_START)):
                XPc = xpool.tile([128, n, 3, PX], BF16, tag=f"xp{c}")
                nc.sync.dma_start(out=XPc, in_=xp_d[:, s:s + n])
                xp_tiles.append(XPc)
            XT = xpool.tile([32, N_UNITS, PX], BF16, tag="xt")
            nc.sync.dma_start(out=XT, in_=xt_d)

            def fview(u, g):
                for c, (n, s) in enumerate(zip(CHUNKS, CHUNK_START)):
                    if s <= u < s + n:
                        return xp_tiles[c][:, u - s, g, :]
                raise AssertionError(u)

            # Per-tag rotating state handled by tile pools; python dicts keep
            # the handles of in-flight tiles across pipeline rounds.
            live = {}
            ochunk = [None]

            def st_x1(u):
                X1 = psum.tile([128, PX], F32, tag="X1", bufs=2)
                mm(X1, CM1, fview(u, 0))
                live[("X1", u)] = X1

            def st_p1(u):
                X1 = live.pop(("X1", u))
                P1a = ppool.tile([128, PX], BF16, tag="P1a")
                stt(out=P1a, in0=X1, scalar=0.0, in1=fview(u, 1), op0=MAX_, op1=MULT)
                P1b = ppool.tile([128, PX], BF16, tag="P1b")
                stt(out=P1b, in0=X1, scalar=0.0, in1=fview(u, 1), op0=MIN_, op1=MULT)
                live[("P1", u)] = (P1a, P1b)

            def st_x2(u):
                P1a, P1b = live.pop(("P1", u))
                X2a = psum.tile([128, PX], F32, tag="X2a", bufs=1)
                mm(X2a, CM1, P1a)
                X2b = psum.tile([128, PX], F32, tag="X2b", bufs=1)
                mm(X2b, CM1, P1b)
                live[("X2", u)] = (X2a, X2b)

            def st_p2(u):
                X2a, X2b = live.pop(("X2", u))
                P2a = ppool.tile([128, PX], BF16, tag="P2a")
                stt(out=P2a, in0=X2a, scalar=0.0, in1=fview(u, 2), op0=MAX_, op1=MULT)
                P2b = ppool.tile([128, PX], BF16, tag="P2b")
                stt(out=P2b, in0=X2b, scalar=0.0, in1=fview(u, 2), op0=MIN_, op1=MULT)
                live[("P2", u)] = (P2a, P2b)

            def st_x3(u):
                P2a, P2b = live.pop(("P2", u))
                X3 = psum.tile([32, PX], F32, tag="X3", bufs=2)
                mm(X3, C3A, P2a, start=True, stop=False)
                mm(X3, C3BN, P2b, start=False, stop=True)
                live[("X3", u)] = X3

            def st_p3(u):
                X3 = live.pop(("X3", u))
                P3 = ppool.tile([32, PX], BF16, tag="P3")
                stt(out=P3, in0=X3, scalar=0.0, in1=XT[:, u, :], op0=MAX_, op1=MULT)
                live[("P3", u)] = P3

            def st_xo(u):
                P3 = live.pop(("P3", u))
                XO = psum.tile([128, PX], F32, tag="XO", bufs=2)
                mm(XO, G2, P3)
                live[("XO", u)] = XO

            def st_out(u):
                XO = live.pop(("XO", u))
                if u % OUT_CHUNK == 0:
                    ochunk[0] = opool.tile([128, OUT_CHUNK, PX], BF16, tag="O",
                                           name="Ochunk")
                nc.scalar.copy(ochunk[0][:, u % OUT_CHUNK, :], XO)
                if u % OUT_CHUNK == OUT_CHUNK - 1:
                    nc.scalar.dma_start(
                        out=out_d[:, u - (OUT_CHUNK - 1):u + 1], in_=ochunk[0])

            # Software-pipelined rounds: PE never consumes a same-round DVE
            # product. Emit order inside a round keeps the three CM1 matmuls
            # adjacent (X1, X2a, X2b) so the stationary reloads once per round.
            N = N_UNITS
            for r in range(-2, N + 2):
                if 0 <= r + 2 < N:
                    st_x1(r + 2)
                if 0 <= r + 1 < N:
                    st_x2(r + 1)
                if 0 <= r < N:
                    st_x3(r)
                if 0 <= r - 1 < N:
                    st_xo(r - 1)
                if 0 <= r < N:
                    st_p3(r)  # feeds next round's XO; input X3(r) just issued
                if 0 <= r + 1 < N:
                    st_p2(r + 1)
                if 0 <= r + 2 < N:
                    st_p1(r + 2)
                if 0 <= r - 2 < N:
                    st_out(r - 2)
    nc.compile()
    return nc


_PROGRAM_CACHE = {}


def _get_program():
    if "nc" not in _PROGRAM_CACHE:
        _PROGRAM_CACHE["nc"] = _build_program()
    return _PROGRAM_CACHE["nc"]


def _make_in_maps(x_real, x_imag, t_coord):
    cm1, c3a, c3bn, g2 = _build_const_mats(np.asarray(t_coord))
    cmats = np.zeros((128, 320), np.float32)
    cmats[:, 0:128] = cm1
    cmats[:, 128:160] = c3a
    cmats[:, 160:192] = c3bn
    cmats[0:32, 192:320] = g2
    cmats = cmats.astype(NPBF16)
    x_real = np.asarray(x_real)
    x_imag = np.asarray(x_imag)
    in_maps = []
    for core in range(NUM_CORES):
        b = core // 2
        h0 = H_SH * (core % 2)
        xs = np.stack([
            x_real[b, 0:192, h0:h0 + H_SH, :],
            x_imag[b, 0:192, h0:h0 + H_SH, :],
        ])  # [2, 192, H, W]
        # xp[(br, c), u, g, px] = xs[br, g*64 + c, 4u + px//128, px%128]
        xp = np.ascontiguousarray(
            xs.reshape(2, 3, 64, N_UNITS, 4, W_)
            .transpose(0, 2, 3, 1, 4, 5)
            .reshape(128, N_UNITS, 3, PX)).astype(NPBF16)
        x3r = x_real[b, 192:200, h0:h0 + H_SH, :]
        x3i = x_imag[b, 192:200, h0:h0 + H_SH, :]
        xt = np.ascontiguousarray(
            np.stack([x3r, x3r, x3i, x3i])
            .reshape(32, N_UNITS, PX)).astype(NPBF16)
        in_maps.append({"xp": xp, "xt": xt, "cmats": cmats})
    return in_maps


def _assemble(results):
    out = np.empty((2, 4, T_, 128, W_), np.float32)
    for core in range(NUM_CORES):
        b = core // 2
        h0 = H_SH * (core % 2)
        # arr[64*br + T, u, 4j + w...] -> [br, T, h = 4u + j, w]
        arr = results[core]["out"].astype(np.float32)
        out[:, b, :, h0:h0 + H_SH, :] = arr.reshape(2, T_, H_SH, W_)
    return out


def kernel_with_info(x_real, x_imag, t_coord, trace=False):
    nc = _get_program()
    in_maps = _make_in_maps(x_real, x_imag, t_coord)
    res = bass_utils.run_bass_kernel_spmd(
        nc, in_maps, core_ids=list(range(NUM_CORES)), trace=trace)
    return _assemble(res.results), res


def kernel(x_real, x_imag, t_coord):
    out, _ = kernel_with_info(x_real, x_imag, t_coord)
    return out


# revision 6
# speedup vs baseline: 1.4734x; 1.1007x over previous
"""Trainium2 Bass kernel for the LIIF non-parametric per-pixel mini-MLP.

Reference computation (per branch, per pixel p = (b,h,w)):
    channels c of feat reshape to W[head, o, i] with c = head*64 + o*8 + i
    t[T, i] = t_coord[T]  (broadcast over i)
    h = einsum('OI,TI->TO', W0, t);  then for k in 1..3: h = W_k @ relu(h)
    out[T] = h[T, 0]

Algebraic identity: t enters rank-1 in T and relu(s*t) = relu(s)*relu(t) +
relu(-s)*relu(-t) (disjoint support), so every intermediate stays in
span{u, v} with u = relu(t), v = relu(-t):
    s0[i]  = sum_j W0[i, j]
    a1 = relu(s0),            b1 = relu(-s0)
    a2 = relu(W1 @ a1),       b2 = relu(W1 @ b1)
    a3 = relu(W2 @ a2),       b3 = relu(W2 @ b2)
    alpha = W3[0, :] . a3,    beta = W3[0, :] . b3
    out[T] = alpha * u[T] + beta * v[T]
Only channels 0:200 of 256 are needed (row 0 of W3).

On-chip mapping (per unit = 512 pixels x both branches; [partition, free]):
    F0/F1/F2 [128, 512] views of a big SBUF tensor (chunked bulk DMA):
        partitions 0:64 = x_real channel slots, 64:128 = x_imag;
        channel g*64 + p%64 at group g.
    X1 = CM1^T @ F0      (PE)   s0 replicated to all (o,i) slots, both branches
    P1a = max(X1,0)*F1   (DVE)  fused relu+mult
    P1b = min(X1,0)*F1   (DVE)  stored = -true_P1b
    X2a = CM1^T @ P1a, X2b = CM1^T @ P1b   (PE; X2b stored = -true)
    P2a = max(X2a,0)*F2, P2b = min(X2b,0)*F2  (DVE; P2b stored = -true)
    X3[32,512] = C3A^T @ P2a + C3BN^T @ P2b   (PE accum; C3BN = -C3B fixes sign)
    P3 = max(X3,0)*F34   (DVE)  F34 = W3row0 repeated [re,re,im,im]
    XO[128,512] = G2^T @ P3    (PE)  partitions = (branch, T), rank-2 expansion
    O = copy(XO) -> bf16 SBUF (ACT), chunked DMA out.

All matmul operands are bf16 (PE runs 1 cycle/column vs 4 for fp32) and all
HBM traffic is bf16 (host converts); PSUM stays fp32. Every stationary except
CM1N is eliminated so the per-unit weight sequence is CM1,CM1,CM1,C3A,C3BN,G2.

The PE instruction stream is software-pipelined 3 rounds deep
(X1(r+2), X2ab(r+1), X3ab(r), XO(r-1)) so no matmul ever waits on a DVE
result produced in the same round.

Sharding: 8 cores, core k -> batch b = k//2, h-half = k%2 (64 h-rows each).
"""

import os

import numpy as np
import concourse.bass as bass
import concourse.bacc as bacc
import concourse.tile as tile
from concourse import mybir
from concourse import bass_utils

F32 = mybir.dt.float32
BF16 = mybir.dt.float16  # fp16: same PE/DVE/DMA cost as bf16, 8x finer mantissa
NPBF16 = np.dtype(np.float16)

NUM_CORES = 8
H_SH = 64             # h rows per core
W_ = 128
T_ = 64
N_UNITS = 16          # units per core; each unit covers 4 h rows = 512 px
PX = 512              # pixels per unit
CHUNKS = [1, 1, 2, 4, 4, 4]       # units per input-DMA chunk (ramp-friendly)
CHUNK_START = [0, 1, 2, 4, 8, 12]
OUT_CHUNK = 2                     # units per output DMA


def _build_const_mats(t_coord: np.ndarray):
    """Host-side constant matrices (tiny, derived from fixed structure + t_coord)."""
    # M1[k = 8i+j, m = 8o+i] = 1 : rep-reduce within one branch block
    m1 = np.zeros((64, 64), np.float32)
    for o in range(8):
        for i in range(8):
            for j in range(8):
                m1[8 * i + j, 8 * o + i] = 1.0
    cm1 = np.zeros((128, 128), np.float32)
    cm1[0:64, 0:64] = m1
    cm1[64:128, 64:128] = m1

    # C3A/C3BN [128, 32]: reduce products to X3 rows [a_re, b_re, a_im, b_im].
    # C3BN carries a -1 so the sign-inverted b-stream (stored = -true) lands
    # with the correct sign in the accumulated X3.
    c3a = np.zeros((128, 32), np.float32)
    c3bn = np.zeros((128, 32), np.float32)
    for i in range(8):
        for j in range(8):
            c3a[8 * i + j, i] = 1.0              # a3_re from P2a re-half
            c3a[64 + 8 * i + j, 16 + i] = 1.0    # a3_im from P2a im-half
            c3bn[8 * i + j, 8 + i] = -1.0        # b3_re from P2b re-half
            c3bn[64 + 8 * i + j, 24 + i] = -1.0  # b3_im from P2b im-half

    # G2 [32, 128]: rank-2 expansion. row 8*(2*br + s) + i, col 64*br + T
    t = t_coord.astype(np.float32)
    u = np.maximum(t, 0.0)
    v = np.maximum(-t, 0.0)
    g2 = np.zeros((32, 128), np.float32)
    for br in range(2):
        for i in range(8):
            g2[8 * (2 * br + 0) + i, 64 * br:64 * (br + 1)] = u
            g2[8 * (2 * br + 1) + i, 64 * br:64 * (br + 1)] = v
    return cm1, c3a, c3bn, g2


def _build_program():
    nc = bacc.Bacc("TRN2", target_bir_lowering=False, debug=False,
                   enable_asserts=False)
    # xp[p, u, g, px] = x[br, g*64 + c, 4u + px//128, px%128], p = 64*br + c
    xp_d = nc.dram_tensor("xp", [128, N_UNITS, 3, PX], BF16, kind="ExternalInput").ap()
    # xt[q, u, px]: q = 16*br_pair + 8*dup + c for channels 192:200, [re,re,im,im]
    xt_d = nc.dram_tensor("xt", [32, N_UNITS, PX], BF16, kind="ExternalInput").ap()
    cmats_d = nc.dram_tensor("cmats", [128, 320], BF16, kind="ExternalInput").ap()
    out_d = nc.dram_tensor("out", [128, N_UNITS, PX], BF16, kind="ExternalOutput").ap()

    MAX_ = mybir.AluOpType.max
    MIN_ = mybir.AluOpType.min
    MULT = mybir.AluOpType.mult

    mm = nc.tensor.matmul
    stt = nc.vector.scalar_tensor_tensor
    stt_b = (nc.vector.scalar_tensor_tensor if os.environ.get("KERNEL_NO_GPSIMD")
             else nc.gpsimd.scalar_tensor_tensor)

    with tile.TileContext(nc) as tc:
        with (
            tc.tile_pool(name="consts", bufs=1) as consts,
            tc.tile_pool(name="xpool", bufs=1) as xpool,
            tc.tile_pool(name="ppool", bufs=2) as ppool,
            tc.tile_pool(name="opool", bufs=2) as opool,
            tc.tile_pool(name="psum", bufs=1, space="PSUM") as psum,
        ):
            CT = consts.tile([128, 320], BF16)
            nc.sync.dma_start(out=CT, in_=cmats_d)
            CM1 = CT[:, 0:128]
            C3A = CT[:, 128:160]
            C3BN = CT[:, 160:192]
            G2 = CT[0:32, 192:320]

            xp_tiles = []
            XT = None
            for c, (n, s) in enumerate(zip(CHUNKS, CHUNK_START)):
                XPc = xpool.tile([128, n, 3, PX], BF16, tag=f"xp{c}")
                nc.sync.dma_start(out=XPc, in_=xp_d[:, s:s + n])
                xp_tiles.append(XPc)
                if c == 0:
                    # xt is small and needed from round 0 on; load it right
                    # after the first xp chunk so P3 never stalls on it.
                    XT = xpool.tile([32, N_UNITS, PX], BF16, tag="xt")
                    nc.sync.dma_start(out=XT, in_=xt_d)

            def fview(u, g):
                for c, (n, s) in enumerate(zip(CHUNKS, CHUNK_START)):
                    if s <= u < s + n:
                        return xp_tiles[c][:, u - s, g, :]
                raise AssertionError(u)

            # Per-tag rotating state handled by tile pools; python dicts keep
            # the handles of in-flight tiles across pipeline rounds.
            live = {}
            ochunk = [None]

            def st_x1(u):
                X1 = psum.tile([128, PX], F32, tag="X1", bufs=2)
                mm(X1, CM1, fview(u, 0))
                live[("X1", u)] = X1

            def st_p1(u):
                X1 = live.pop(("X1", u))
                P1a = ppool.tile([128, PX], BF16, tag="P1a")
                stt(out=P1a, in0=X1, scalar=0.0, in1=fview(u, 1), op0=MAX_, op1=MULT)
                P1b = ppool.tile([128, PX], BF16, tag="P1b")
                stt_b(out=P1b, in0=X1, scalar=0.0, in1=fview(u, 1), op0=MIN_, op1=MULT)
                live[("P1", u)] = (P1a, P1b)

            def st_x2(u):
                P1a, P1b = live.pop(("P1", u))
                X2a = psum.tile([128, PX], F32, tag="X2a", bufs=1)
                mm(X2a, CM1, P1a)
                X2b = psum.tile([128, PX], F32, tag="X2b", bufs=1)
                mm(X2b, CM1, P1b)
                live[("X2", u)] = (X2a, X2b)

            def st_p2(u):
                X2a, X2b = live.pop(("X2", u))
                P2a = ppool.tile([128, PX], BF16, tag="P2a")
                stt(out=P2a, in0=X2a, scalar=0.0, in1=fview(u, 2), op0=MAX_, op1=MULT)
                P2b = ppool.tile([128, PX], BF16, tag="P2b")
                stt_b(out=P2b, in0=X2b, scalar=0.0, in1=fview(u, 2), op0=MIN_, op1=MULT)
                live[("P2", u)] = (P2a, P2b)

            def st_x3(u):
                P2a, P2b = live.pop(("P2", u))
                X3 = psum.tile([32, PX], F32, tag="X3", bufs=2)
                mm(X3, C3A, P2a, start=True, stop=False)
                mm(X3, C3BN, P2b, start=False, stop=True)
                live[("X3", u)] = X3

            def st_p3(u):
                X3 = live.pop(("X3", u))
                P3 = ppool.tile([32, PX], BF16, tag="P3")
                stt(out=P3, in0=X3, scalar=0.0, in1=XT[:, u, :], op0=MAX_, op1=MULT)
                live[("P3", u)] = P3

            def st_xo(u):
                P3 = live.pop(("P3", u))
                XO = psum.tile([128, PX], F32, tag="XO", bufs=2)
                mm(XO, G2, P3)
                live[("XO", u)] = XO

            def st_out(u):
                XO = live.pop(("XO", u))
                if u % OUT_CHUNK == 0:
                    ochunk[0] = opool.tile([128, OUT_CHUNK, PX], BF16, tag="O",
                                           name="Ochunk")
                nc.scalar.copy(ochunk[0][:, u % OUT_CHUNK, :], XO)
                if u % OUT_CHUNK == OUT_CHUNK - 1:
                    nc.scalar.dma_start(
                        out=out_d[:, u - (OUT_CHUNK - 1):u + 1], in_=ochunk[0])

            # Software-pipelined rounds: PE never consumes a same-round DVE
            # product. Emit order inside a round keeps the three CM1 matmuls
            # adjacent (X1, X2a, X2b) so the stationary reloads once per round.
            N = N_UNITS
            for r in range(-2, N + 2):
                if 0 <= r + 2 < N:
                    st_x1(r + 2)
                if 0 <= r + 1 < N:
                    st_x2(r + 1)
                if 0 <= r < N:
                    st_x3(r)
                if 0 <= r - 1 < N:
                    st_xo(r - 1)
                if 0 <= r < N:
                    st_p3(r)  # feeds next round's XO; input X3(r) just issued
                if 0 <= r + 1 < N:
                    st_p2(r + 1)
                if 0 <= r + 2 < N:
                    st_p1(r + 2)
                if 0 <= r - 2 < N:
                    st_out(r - 2)
    nc.compile()
    return nc


_PROGRAM_CACHE = {}


def _get_program():
    if "nc" not in _PROGRAM_CACHE:
        _PROGRAM_CACHE["nc"] = _build_program()
    return _PROGRAM_CACHE["nc"]


def _make_in_maps(x_real, x_imag, t_coord):
    cm1, c3a, c3bn, g2 = _build_const_mats(np.asarray(t_coord))
    cmats = np.zeros((128, 320), np.float32)
    cmats[:, 0:128] = cm1
    cmats[:, 128:160] = c3a
    cmats[:, 160:192] = c3bn
    cmats[0:32, 192:320] = g2
    cmats = cmats.astype(NPBF16)
    x_real = np.asarray(x_real)
    x_imag = np.asarray(x_imag)
    in_maps = []
    for core in range(NUM_CORES):
        b = core // 2
        h0 = H_SH * (core % 2)
        xs = np.stack([
            x_real[b, 0:192, h0:h0 + H_SH, :],
            x_imag[b, 0:192, h0:h0 + H_SH, :],
        ])  # [2, 192, H, W]
        # xp[(br, c), u, g, px] = xs[br, g*64 + c, 4u + px//128, px%128]
        xp = np.ascontiguousarray(
            xs.reshape(2, 3, 64, N_UNITS, 4, W_)
            .transpose(0, 2, 3, 1, 4, 5)
            .reshape(128, N_UNITS, 3, PX)).astype(NPBF16)
        x3r = x_real[b, 192:200, h0:h0 + H_SH, :]
        x3i = x_imag[b, 192:200, h0:h0 + H_SH, :]
        xt = np.ascontiguousarray(
            np.stack([x3r, x3r, x3i, x3i])
            .reshape(32, N_UNITS, PX)).astype(NPBF16)
        in_maps.append({"xp": xp, "xt": xt, "cmats": cmats})
    return in_maps


def _assemble(results):
    out = np.empty((2, 4, T_, 128, W_), np.float32)
    for core in range(NUM_CORES):
        b = core // 2
        h0 = H_SH * (core % 2)
        # arr[64*br + T, u, 4j + w...] -> [br, T, h = 4u + j, w]
        arr = results[core]["out"].astype(np.float32)
        out[:, b, :, h0:h0 + H_SH, :] = arr.reshape(2, T_, H_SH, W_)
    return out


def kernel_with_info(x_real, x_imag, t_coord, trace=False):
    nc = _get_program()
    in_maps = _make_in_maps(x_real, x_imag, t_coord)
    res = bass_utils.run_bass_kernel_spmd(
        nc, in_maps, core_ids=list(range(NUM_CORES)), trace=trace)
    return _assemble(res.results), res


def kernel(x_real, x_imag, t_coord):
    out, _ = kernel_with_info(x_real, x_imag, t_coord)
    return out
